# revision 15
# baseline (speedup 1.0000x reference)
"""Two-layer GCN encoder on 8 TRN2 NeuronCores.

Strategy (graph/data parallel, dst-sharded), v2:
  - Nodes partitioned contiguously across 8 cores (6250 dst rows each).
  - Features, weights, scatter matrices all bf16 (fp32 PSUM accumulate):
    halves gather/DMA traffic and quadruples PE throughput vs fp32.
  - Per layer:  agg.T[f, d] = sum_e gathered_feat[e, f] * M[e, d]  via PE
    matmuls over 128-edge chunks; M holds the GCN edge norm at the edge's
    local dst column (self-loops as explicit edges).  out.T = W.T @ agg.T,
    bias(/relu) epilogue, PE transpose back to row-major.
  - TILE=128 dst rows per PSUM tile (49 tiles/core), GROUP=8 tiles per
    gather batch.
  - M is built ON-CHIP (2 DVE ops per group): cmp = (iota == dloc_bcast),
    mt = cmp * norm_bcast, from tiny per-chunk dloc/norm blobs instead of
    DMAing the dense [128, C, TILE] scatter matrix from HBM.
  - Epilogue rows are batched per group: one store DMA per group instead of
    per tile (SP engine was 87% busy on per-tile stores).
  - Layer-1 activations are stored bf16 and AllGathered so every core holds
    the full feature table for layer 2's gathers.
  - dma_gather indices are int16, so each (tile, src-half) segment gathers
    from base row 0 or row 32768 of the feature table.
"""

import os
import sys
import numpy as np

for _p in ("/opt/trn_rl_repo", "/root/.axon_site/_ro/trn_rl_repo"):
    if os.path.isdir(_p) and _p not in sys.path:
        sys.path.insert(0, _p)

N = 50000
D = 128
CORES = 8
NPC = N // CORES            # 6250 dst rows per core
TILE = 128                  # dst rows per psum tile
NT = (NPC + TILE - 1) // TILE   # 49 tiles per core (last tile has 106 rows)
LAST_ROWS = NPC - (NT - 1) * TILE
SPLIT = 32768               # int16 gather-index base split
BLK = 32                    # dst columns per scatter-matmul block
NBLK = TILE // BLK          # 4 column blocks per psum tile
GROUP = 4                   # tiles per gather batch
MAXC = 8                    # chunks per dma_gather call (1024-descriptor ring)
NQ = 4                      # SWDGE queues used round-robin for gathers


def _prep(edge_index):
    """Sort/pad edges; build per-core gather-index and dcol/norm blobs.

    Segments are per (tile128, lane, 32-dst block): the chunk matmul
    accumulates into a 32-column slice of the [128,128] psum tile, so the
    SBUF-resident M matrix is only [128, C, 32]."""
    src = np.asarray(edge_index[0], dtype=np.int64)
    dst = np.asarray(edge_index[1], dtype=np.int64)
    deg = (np.bincount(dst, minlength=N) + 1).astype(np.float32)
    dinv = (1.0 / np.sqrt(deg)).astype(np.float32)

    loop = np.arange(N, dtype=np.int64)
    s_all = np.concatenate([src, loop])
    d_all = np.concatenate([dst, loop])
    norm = dinv[s_all] * dinv[d_all]

    core = d_all // NPC
    lcl = d_all - core * NPC
    t = lcl // TILE
    dloc = lcl - t * TILE
    blk = dloc // BLK
    dcol = dloc - blk * BLK
    lane = (s_all >= SPLIT).astype(np.int64)
    # key encodes (core, tile, lane, blk)
    key = ((core * NT + t) * 2 + lane) * NBLK + blk

    order = np.argsort(key, kind="stable")
    key_s = key[order]
    s_s = s_all[order]
    norm_s = norm[order]
    dcol_s = dcol[order]
    lane_s = lane[order]
    core_s = core[order]

    counts = np.bincount(key, minlength=CORES * NT * 2 * NBLK)
    counts = counts.reshape(CORES, NT, 2, NBLK)
    # uniform across cores; ensure >=1 chunk per (tile, blk) so psum cols
    # are always written (pad chunks have norm=0)
    segchunks = ((counts + 127) // 128).max(axis=0)  # [NT, 2, NBLK]
    empty = segchunks.sum(axis=1) == 0               # [NT, NBLK]
    segchunks[:, 0, :][empty] = 1

    # chunk order: per group of GROUP tiles, all lo segments then all hi
    # segments (tile-major, blk-minor within each lane)
    n_groups = (NT + GROUP - 1) // GROUP
    groups = []          # list of dicts with static layout info
    seg_chunk_start = np.zeros((NT, 2, NBLK), dtype=np.int64)
    c = 0
    for g in range(n_groups):
        ts = list(range(g * GROUP, min((g + 1) * GROUP, NT)))
        c0 = c
        for tt in ts:
            for bb in range(NBLK):
                seg_chunk_start[tt, 0, bb] = c
                c += segchunks[tt, 0, bb]
        glo = c - c0
        for tt in ts:
            for bb in range(NBLK):
                seg_chunk_start[tt, 1, bb] = c
                c += segchunks[tt, 1, bb]
        ghi = c - c0 - glo
        groups.append({"tiles": ts, "c0": c0, "glo": glo, "ghi": ghi})
    C_total = c
    S = C_total * 128

    # slot of each edge inside its core's blob
    key_starts = np.zeros(CORES * NT * 2 * NBLK + 1, dtype=np.int64)
    np.cumsum(counts.reshape(-1), out=key_starts[1:])
    rank = np.arange(len(key_s)) - key_starts[key_s]
    seg_slot_start = seg_chunk_start * 128  # [NT, 2, NBLK]
    k_core = key_s // (NT * 2 * NBLK)
    k_rem = key_s - k_core * (NT * 2 * NBLK)
    t_s = k_rem // (2 * NBLK)
    l_s = (k_rem // NBLK) % 2
    b_s = k_rem % NBLK
    slot = seg_slot_start[t_s, l_s, b_s] + rank

    idx_val = (s_s - lane_s * SPLIT).astype(np.int16)
    idx_flat = np.zeros((CORES, S), dtype=np.int16)
    idx_flat[core_s, slot] = idx_val
    # per-slot dcol / norm (padding slots keep dcol=0, norm=0 -> M row = 0)
    dcol_flat = np.zeros((CORES, S), dtype=np.float32)
    dcol_flat[core_s, slot] = dcol_s
    norm_flat = np.zeros((CORES, S), dtype=np.float32)
    norm_flat[core_s, slot] = norm_s

    # idx i lives at partition i%16 (replicated x8 across the 128 partitions)
    idx_arr = idx_flat.reshape(CORES, S // 16, 16).transpose(0, 2, 1)
    idx_arr = np.tile(idx_arr, (1, 8, 1)).copy()          # [CORES, 128, S//16]
    # dcol/norm blobs: [CORES, 128 (edge slot in chunk), C_total]
    import jax.numpy as jnp
    dcol_arr = np.asarray(
        jnp.asarray(dcol_flat.reshape(CORES, C_total, 128).transpose(0, 2, 1),
                    ).astype(jnp.bfloat16))
    norm_arr = norm_flat.reshape(CORES, C_total, 128).transpose(0, 2, 1).copy()

    # per-(tile, blk): (global chunk list, group-local chunk list)
    tile_chunks = {}
    for g in groups:
        for tt in g["tiles"]:
            for bb in range(NBLK):
                lo0 = seg_chunk_start[tt, 0, bb]
                hi0 = seg_chunk_start[tt, 1, bb]
                cg = (list(range(lo0, lo0 + segchunks[tt, 0, bb]))
                      + list(range(hi0, hi0 + segchunks[tt, 1, bb])))
                tile_chunks[(tt, bb)] = cg

    return {
        "groups": groups,
        "tile_chunks": tile_chunks,
        "segchunks": segchunks,
        "C_total": C_total,
        "S": S,
        "idx_arr": idx_arr,
        "dcol_arr": dcol_arr,
        "norm_arr": norm_arr,
    }


# ablation switches for performance bisection (all True in production)
_FLAGS = {"gather": True, "mbuild": True, "mm": True, "epi": True, "cc": True}
# body replication count (timing only; >1 repeats the whole kernel in one NEFF)
_REPS = 1


def _build(meta):
    import concourse.bacc as bacc
    import concourse.mybir as mybir
    import concourse.tile as tile

    f32 = mybir.dt.float32
    bf16 = mybir.dt.bfloat16
    i16 = mybir.dt.int16

    C_total = meta["C_total"]
    S = meta["S"]
    groups = meta["groups"]
    tile_chunks = meta["tile_chunks"]

    nc = bacc.Bacc("TRN2", target_bir_lowering=False, debug=False,
                   enable_asserts=True, num_devices=CORES,
                   num_swdge_queues=NQ)

    xf = nc.dram_tensor("xf", [N, D], bf16, kind="ExternalInput")
    dlocb = nc.dram_tensor("dlocb", [128, C_total], bf16, kind="ExternalInput")
    normb = nc.dram_tensor("normb", [128, C_total], f32, kind="ExternalInput")
    idxb = nc.dram_tensor("idxb", [128, S // 16], i16, kind="ExternalInput")
    w1 = nc.dram_tensor("w1", [D, D], bf16, kind="ExternalInput")
    w2 = nc.dram_tensor("w2", [D, D], bf16, kind="ExternalInput")
    b1 = nc.dram_tensor("b1", [D, 1], f32, kind="ExternalInput")
    b2 = nc.dram_tensor("b2", [D, 1], f32, kind="ExternalInput")
    id32 = nc.dram_tensor("id32", [128, 128], f32, kind="ExternalInput")
    h1loc = nc.dram_tensor("h1loc", [NPC, D], bf16, kind="Internal")
    h1full = nc.dram_tensor("h1full", [N, D], bf16, kind="Internal",
                            addr_space="Shared")
    outp = nc.dram_tensor("outp", [NPC, D], f32, kind="ExternalOutput")

    gmax = max(g["glo"] + g["ghi"] for g in groups)
    qctr = [0]

    with tile.TileContext(nc) as tc:
        with (
            tc.tile_pool(name="const", bufs=1) as cpool,
            tc.tile_pool(name="gath", bufs=2) as gpool,
            tc.tile_pool(name="small", bufs=4) as spool,
            tc.tile_pool(name="rows", bufs=2) as rpool,
            tc.tile_pool(name="agg_ps", bufs=2, space="PSUM") as agg_ps,
            tc.tile_pool(name="out_ps", bufs=2, space="PSUM") as out_ps,
            tc.tile_pool(name="tr_ps", bufs=2, space="PSUM") as tr_ps,
        ):
            idx_t = cpool.tile([128, S // 16], i16, tag="idx")
            nc.sync.dma_start(idx_t[:], idxb.ap())
            dloc_t = cpool.tile([128, C_total], bf16, tag="dloc")
            nc.sync.dma_start(dloc_t[:], dlocb.ap())
            norm_t = cpool.tile([128, C_total], f32, tag="norm")
            nc.sync.dma_start(norm_t[:], normb.ap())
            w1_t = cpool.tile([D, D], bf16, tag="w1")
            nc.sync.dma_start(w1_t[:], w1.ap())
            w2_t = cpool.tile([D, D], bf16, tag="w2")
            nc.sync.dma_start(w2_t[:], w2.ap())
            b1_t = cpool.tile([D, 1], f32, tag="b1")
            nc.sync.dma_start(b1_t[:], b1.ap())
            b2_t = cpool.tile([D, 1], f32, tag="b2")
            nc.sync.dma_start(b2_t[:], b2.ap())
            id32_t = cpool.tile([128, 128], f32, tag="id32")
            nc.sync.dma_start(id32_t[:], id32.ap())
            # iota[p, 0, d] = d   (bf16; values < 32 are exact)
            iota_t = cpool.tile([128, 1, BLK], bf16, tag="iota")
            nc.gpsimd.iota(iota_t[:], pattern=[[0, 1], [1, BLK]],
                           channel_multiplier=0,
                           allow_small_or_imprecise_dtypes=True)
            # SBUF-resident scatter matrix, built once (shared by both layers):
            # m_t[p, c, d] = (d == dcol[p, c]) * norm[p, c]
            m_t = cpool.tile([128, C_total, BLK], bf16, tag="mres")
            nc.vector.tensor_tensor(
                m_t[:],
                iota_t[:, 0:1, :].broadcast_to([128, C_total, BLK]),
                dloc_t[:, :].unsqueeze(2).broadcast_to([128, C_total, BLK]),
                op=mybir.AluOpType.is_equal)
            nc.vector.tensor_tensor(
                m_t[:], m_t[:],
                norm_t[:, :].unsqueeze(2).broadcast_to([128, C_total, BLK]),
                op=mybir.AluOpType.mult)

            for _rep in range(_REPS):
              for layer in (1, 2):
                feat = xf if layer == 1 else h1full
                w_t = w1_t if layer == 1 else w2_t
                for g in groups:
                    glo, ghi = g["glo"], g["ghi"]
                    G = glo + ghi
                    c0 = g["c0"]
                    ntiles = len(g["tiles"])
                    gt = gpool.tile([128, gmax, D], bf16, tag="gt")
                    for lane, nch, base in ((0, glo, feat.ap()),
                                            (1, ghi, feat.ap()[SPLIT:N, :])):
                        if not _FLAGS["gather"]:
                            break
                        off = 0 if lane == 0 else glo
                        for cs in range(0, nch, MAXC):
                            cw = min(MAXC, nch - cs)
                            a = off + cs
                            nc.gpsimd.dma_gather(
                                gt[:, a:a + cw, :], base,
                                idx_t[:, (c0 + a) * 8:(c0 + a + cw) * 8],
                                num_idxs=cw * 128, num_idxs_reg=cw * 128,
                                elem_size=D,
                                queue_num=qctr[0] % NQ)
                            qctr[0] += 1
                    rowt = rpool.tile([128, GROUP, D],
                                      bf16 if layer == 1 else f32,
                                      tag="hro" if layer == 1 else "oro")
                    for ti, tt in enumerate(g["tiles"]):
                        ps = agg_ps.tile([D, TILE], f32, tag="agg")
                        if _FLAGS["mm"]:
                            for bb in range(NBLK):
                                chunks = tile_chunks[(tt, bb)]
                                pcol = ps[:, bb * BLK:(bb + 1) * BLK]
                                for k, cg in enumerate(chunks):
                                    nc.tensor.matmul(
                                        pcol, gt[:, cg - c0, :],
                                        m_t[:, cg, :],
                                        start=(k == 0),
                                        stop=(k == len(chunks) - 1))
                        else:
                            for bb in range(NBLK):
                                nc.tensor.matmul(
                                    ps[:, bb * BLK:(bb + 1) * BLK],
                                    gt[:, 0, :], m_t[:, 0, :],
                                    start=True, stop=True)
                        aggT = spool.tile([D, TILE], bf16, tag="aggT")
                        nc.vector.tensor_copy(aggT[:], ps[:])
                        po = out_ps.tile([D, TILE], f32, tag="po")
                        nc.tensor.matmul(po[:], w_t[:], aggT[:],
                                         start=True, stop=True)
                        if layer == 1:
                            hT = spool.tile([D, TILE], f32, tag="hT")
                            nc.scalar.activation(
                                hT[:], po[:],
                                mybir.ActivationFunctionType.Relu,
                                bias=b1_t[:, 0:1], scale=1.0)
                            pt = tr_ps.tile([TILE, D], f32, tag="pt")
                            nc.tensor.transpose(pt[:], hT[:], id32_t[:])
                            nc.vector.tensor_copy(rowt[:, ti, :], pt[:])
                        else:
                            oT = spool.tile([D, TILE], f32, tag="oT")
                            nc.vector.tensor_scalar_add(oT[:], po[:],
                                                        b2_t[:, 0:1])
                            pt = tr_ps.tile([TILE, D], f32, tag="pt")
                            nc.tensor.transpose(pt[:], oT[:], id32_t[:])
                            nc.vector.tensor_copy(rowt[:, ti, :], pt[:])
                    # one batched store per group
                    r0 = g["tiles"][0] * TILE
                    rows = sum(TILE if tt < NT - 1 else LAST_ROWS
                               for tt in g["tiles"])
                    dstt = h1loc if layer == 1 else outp
                    srct = rowt
                    if rows == ntiles * TILE:
                        dap = dstt.ap()[r0:r0 + rows, :].rearrange(
                            "(t p) f -> p t f", p=TILE)
                        nc.sync.dma_start(dap, srct[:, 0:ntiles, :])
                    else:
                        # last group: full tiles batched, partial tile alone
                        nfull = ntiles - 1
                        if nfull:
                            dap = dstt.ap()[r0:r0 + nfull * TILE, :].rearrange(
                                "(t p) f -> p t f", p=TILE)
                            nc.sync.dma_start(dap, srct[:, 0:nfull, :])
                        pr0 = r0 + nfull * TILE
                        nc.sync.dma_start(
                            dstt.ap()[pr0:pr0 + LAST_ROWS, :],
                            srct[0:LAST_ROWS, nfull, :])
                if layer == 1 and _FLAGS["cc"]:
                    nc.gpsimd.collective_compute(
                        "AllGather", mybir.AluOpType.bypass,
                        replica_groups=[list(range(CORES))],
                        ins=[h1loc.ap()], outs=[h1full.ap()])
    nc.compile()
    return nc


class _Exec:
    """Device-resident SPMD executor mirroring bass2jax.run_bass_via_pjrt's
    multi-core branch, but caching the jitted callable and the device-resident
    input arrays so repeated runs skip re-trace and host->device transfer."""

    def __init__(self, nc):
        import jax
        import numpy as _np
        import concourse.mybir as mybir
        from concourse import bass2jax
        from jax.experimental.shard_map import shard_map
        from jax.sharding import Mesh, PartitionSpec

        bass2jax.install_neuronx_cc_hook()
        self.jax = jax
        self.nc = nc
        in_names, out_names, out_avals, zero_outs = [], [], [], []
        partition_name = (nc.partition_id_tensor.name
                          if nc.partition_id_tensor else None)
        for alloc in nc.m.functions[0].allocations:
            if not isinstance(alloc, mybir.MemoryLocationSet):
                continue
            name = alloc.memorylocations[0].name
            if alloc.kind == "ExternalInput":
                if name != partition_name:
                    in_names.append(name)
            elif alloc.kind == "ExternalOutput":
                out_names.append(name)
                shape = tuple(alloc.tensor_shape)
                dtype = mybir.dt.np(alloc.dtype)
                out_avals.append(jax.core.ShapedArray(shape, dtype))
                zero_outs.append(_np.zeros(shape, dtype))
        self.in_names, self.out_names = in_names, out_names
        self.out_avals, self.zero_outs = out_avals, zero_outs
        n_params, n_outs = len(in_names), len(out_names)
        all_names = list(in_names) + list(out_names)
        if partition_name is not None:
            all_names.append(partition_name)

        def _body(*args):
            ins = list(args[:n_params])
            outs = list(args[n_params:])
            operands = ins + outs
            if partition_name is not None:
                operands.append(bass2jax.partition_id_tensor())
            outs = list(bass2jax._bass_exec_p.bind(
                *operands,
                out_avals=tuple(out_avals),
                in_names=tuple(all_names),
                out_names=tuple(out_names),
                lowering_input_output_aliases=(),
                sim_require_finite=True,
                sim_require_nnan=True,
                nc=nc,
            ))
            return tuple(outs)

        devices = jax.devices()[:CORES]
        mesh = Mesh(_np.asarray(devices), ("core",))
        in_specs = (PartitionSpec("core"),) * (n_params + n_outs)
        out_specs = (PartitionSpec("core"),) * n_outs
        self.mesh = mesh
        self.sharded = jax.jit(
            shard_map(_body, mesh=mesh, in_specs=in_specs,
                      out_specs=out_specs, check_rep=False),
            donate_argnums=tuple(range(n_params, n_params + n_outs)),
            keep_unused=True,
        )
        self.dev_in = None

    def upload(self, in_maps):
        import jax
        import numpy as _np
        from jax.sharding import NamedSharding, PartitionSpec
        concat_in = [
            _np.concatenate([_np.asarray(in_maps[c][nm]) for c in range(CORES)],
                            axis=0)
            for nm in self.in_names
        ]
        sh = NamedSharding(self.mesh, PartitionSpec("core"))
        self.dev_in = [jax.device_put(a, sh) for a in concat_in]
        for a in self.dev_in:
            a.block_until_ready()

    def _zeros(self):
        import jax
        import numpy as _np
        from jax.sharding import NamedSharding, PartitionSpec
        sh = NamedSharding(self.mesh, PartitionSpec("core"))
        return [
            jax.device_put(
                _np.zeros((CORES * z.shape[0], *z.shape[1:]), z.dtype), sh)
            for z in self.zero_outs
        ]

    def run(self):
        import numpy as _np
        outs = self.sharded(*self.dev_in, *self._zeros())
        res = []
        for i, nm in enumerate(self.out_names):
            a = _np.asarray(outs[i]).reshape(CORES, *self.out_avals[i].shape)
            res.append(a)
        return dict(zip(self.out_names, res))

    def _time_burst(self, k, n):
        """Best wall over n trials of k back-to-back async executions with
        device-resident inputs and pre-uploaded donated output buffers."""
        import time as _t
        times = []
        for _ in range(n):
            zs_list = [self._zeros() for _ in range(k)]
            for zs in zs_list:
                for z in zs:
                    z.block_until_ready()
            t0 = _t.perf_counter()
            outs = [self.sharded(*self.dev_in, *zs) for zs in zs_list]
            for os_ in outs:
                for o in os_:
                    o.block_until_ready()
            times.append(_t.perf_counter() - t0)
        return min(times)


_CACHE = {}


def _in_maps(meta, x, W1, b1, W2, b2):
    import jax.numpy as jnp
    xf = np.asarray(jnp.asarray(np.asarray(x, dtype=np.float32)
                                ).astype(jnp.bfloat16))
    w1f = np.asarray(jnp.asarray(np.asarray(W1, dtype=np.float32)
                                 ).astype(jnp.bfloat16))
    w2f = np.asarray(jnp.asarray(np.asarray(W2, dtype=np.float32)
                                 ).astype(jnp.bfloat16))
    b1f = np.asarray(b1, dtype=np.float32).reshape(D, 1)
    b2f = np.asarray(b2, dtype=np.float32).reshape(D, 1)
    id32 = np.eye(128, dtype=np.float32)
    return [{
        "xf": xf,
        "dlocb": meta["dcol_arr"][c],
        "normb": meta["norm_arr"][c],
        "idxb": meta["idx_arr"][c],
        "w1": w1f, "w2": w2f, "b1": b1f, "b2": b2f,
        "id32": id32,
    } for c in range(CORES)]


def kernel(x, edge_index, W1, b1, W2, b2):
    meta = _prep(edge_index)
    nc = _build(meta)
    ex = _Exec(nc)
    ex.upload(_in_maps(meta, x, W1, b1, W2, b2))
    res = ex.run()
    _CACHE["exec"] = ex
    _CACHE["meta"] = meta
    out = res["outp"].reshape(N, D)
    return out.astype(np.float32)


def bench(n=4):
    """Differential per-exec time in ns: repeat the whole kernel body 5x
    inside a second NEFF and difference against the single-body NEFF, so
    dispatch/tunnel overhead cancels."""
    global _REPS
    meta = _CACHE["meta"]
    ex1 = _CACHE["exec"]
    w1 = ex1._time_burst(1, n + 2)
    old = _REPS
    try:
        _REPS = 5
        nc5 = _build(meta)
        ex5 = _Exec(nc5)
        ex5.dev_in = ex1.dev_in
        w5 = ex5._time_burst(1, n + 2)
    finally:
        _REPS = old
    return (w5 - w1) / 4 * 1e9


# revision 24
# speedup vs baseline: 7.1830x; 7.1830x over previous
"""Two-layer GCN encoder on 8 TRN2 NeuronCores.

Strategy (graph/data parallel, dst-sharded), v2:
  - Nodes partitioned contiguously across 8 cores (6250 dst rows each).
  - Features, weights, scatter matrices all bf16 (fp32 PSUM accumulate):
    halves gather/DMA traffic and quadruples PE throughput vs fp32.
  - Per layer:  agg.T[f, d] = sum_e gathered_feat[e, f] * M[e, d]  via PE
    matmuls over 128-edge chunks; M holds the GCN edge norm at the edge's
    local dst column (self-loops as explicit edges).  out.T = W.T @ agg.T,
    bias(/relu) epilogue, PE transpose back to row-major.
  - TILE=128 dst rows per PSUM tile (49 tiles/core), GROUP=8 tiles per
    gather batch.
  - M is built ON-CHIP (2 DVE ops per group): cmp = (iota == dloc_bcast),
    mt = cmp * norm_bcast, from tiny per-chunk dloc/norm blobs instead of
    DMAing the dense [128, C, TILE] scatter matrix from HBM.
  - Epilogue rows are batched per group: one store DMA per group instead of
    per tile (SP engine was 87% busy on per-tile stores).
  - Layer-1 activations are stored bf16 and AllGathered so every core holds
    the full feature table for layer 2's gathers.
  - dma_gather indices are int16, so each (tile, src-half) segment gathers
    from base row 0 or row 32768 of the feature table.
"""

import os
import sys
import numpy as np

for _p in ("/opt/trn_rl_repo", "/root/.axon_site/_ro/trn_rl_repo"):
    if os.path.isdir(_p) and _p not in sys.path:
        sys.path.insert(0, _p)

N = 50000
D = 128
CORES = 8
NPC = N // CORES            # 6250 dst rows per core
TILE = 128                  # dst rows per psum tile
NT = (NPC + TILE - 1) // TILE   # 49 tiles per core (last tile has 106 rows)
LAST_ROWS = NPC - (NT - 1) * TILE
SPLIT = 32768               # int16 gather-index base split
BLK = 32                    # dst columns per scatter-matmul block
NBLK = TILE // BLK          # 4 column blocks per psum tile
GROUP = 4                   # tiles per gather batch
MAXC = 8                    # chunks per dma_gather call (1024-descriptor ring)
NQ = 4                      # SWDGE queues used round-robin for gathers


def _prep(edge_index):
    """Sort/pad edges; build per-core gather-index and dcol/norm blobs.

    Segments are per (tile128, lane, 32-dst block): the chunk matmul
    accumulates into a 32-column slice of the [128,128] psum tile, so the
    SBUF-resident M matrix is only [128, C, 32]."""
    src = np.asarray(edge_index[0], dtype=np.int64)
    dst = np.asarray(edge_index[1], dtype=np.int64)
    deg = (np.bincount(dst, minlength=N) + 1).astype(np.float32)
    dinv = (1.0 / np.sqrt(deg)).astype(np.float32)

    loop = np.arange(N, dtype=np.int64)
    s_all = np.concatenate([src, loop])
    d_all = np.concatenate([dst, loop])
    norm = dinv[s_all] * dinv[d_all]

    core = d_all // NPC
    lcl = d_all - core * NPC
    t = lcl // TILE
    dloc = lcl - t * TILE
    blk = dloc // BLK
    dcol = dloc - blk * BLK
    lane = (s_all >= SPLIT).astype(np.int64)
    # key encodes (core, tile, lane, blk)
    key = ((core * NT + t) * 2 + lane) * NBLK + blk

    order = np.argsort(key, kind="stable")
    key_s = key[order]
    s_s = s_all[order]
    norm_s = norm[order]
    dcol_s = dcol[order]
    lane_s = lane[order]
    core_s = core[order]

    counts = np.bincount(key, minlength=CORES * NT * 2 * NBLK)
    counts = counts.reshape(CORES, NT, 2, NBLK)
    # uniform across cores; ensure >=1 chunk per (tile, blk) so psum cols
    # are always written (pad chunks have norm=0)
    segchunks = ((counts + 127) // 128).max(axis=0)  # [NT, 2, NBLK]
    empty = segchunks.sum(axis=1) == 0               # [NT, NBLK]
    segchunks[:, 0, :][empty] = 1

    # chunk order: per group of GROUP tiles, all lo segments then all hi
    # segments (tile-major, blk-minor within each lane)
    n_groups = (NT + GROUP - 1) // GROUP
    groups = []          # list of dicts with static layout info
    seg_chunk_start = np.zeros((NT, 2, NBLK), dtype=np.int64)
    c = 0
    for g in range(n_groups):
        ts = list(range(g * GROUP, min((g + 1) * GROUP, NT)))
        c0 = c
        for tt in ts:
            for bb in range(NBLK):
                seg_chunk_start[tt, 0, bb] = c
                c += segchunks[tt, 0, bb]
        glo = c - c0
        for tt in ts:
            for bb in range(NBLK):
                seg_chunk_start[tt, 1, bb] = c
                c += segchunks[tt, 1, bb]
        ghi = c - c0 - glo
        groups.append({"tiles": ts, "c0": c0, "glo": glo, "ghi": ghi})
    C_total = c
    S = C_total * 128

    # slot of each edge inside its core's blob
    key_starts = np.zeros(CORES * NT * 2 * NBLK + 1, dtype=np.int64)
    np.cumsum(counts.reshape(-1), out=key_starts[1:])
    rank = np.arange(len(key_s)) - key_starts[key_s]
    seg_slot_start = seg_chunk_start * 128  # [NT, 2, NBLK]
    k_core = key_s // (NT * 2 * NBLK)
    k_rem = key_s - k_core * (NT * 2 * NBLK)
    t_s = k_rem // (2 * NBLK)
    l_s = (k_rem // NBLK) % 2
    b_s = k_rem % NBLK
    slot = seg_slot_start[t_s, l_s, b_s] + rank

    idx_val = (s_s - lane_s * SPLIT).astype(np.int16)
    # pad slots: spread gather addresses over distinct (valid) rows instead of
    # hammering row 0 — their M entries are 0 so the data is discarded
    idx_flat = np.tile((np.arange(S, dtype=np.int64) % 16384).astype(np.int16),
                       (CORES, 1))
    idx_flat[core_s, slot] = idx_val
    # per-slot dcol / norm (padding slots keep dcol=0, norm=0 -> M row = 0)
    dcol_flat = np.zeros((CORES, S), dtype=np.float32)
    dcol_flat[core_s, slot] = dcol_s
    norm_flat = np.zeros((CORES, S), dtype=np.float32)
    norm_flat[core_s, slot] = norm_s

    # idx i lives at partition i%16 (replicated x8 across the 128 partitions)
    idx_arr = idx_flat.reshape(CORES, S // 16, 16).transpose(0, 2, 1)
    idx_arr = np.tile(idx_arr, (1, 8, 1)).copy()          # [CORES, 128, S//16]
    # dcol/norm blobs: [CORES, 128 (edge slot in chunk), C_total]
    import jax.numpy as jnp
    dcol_arr = np.asarray(
        jnp.asarray(dcol_flat.reshape(CORES, C_total, 128).transpose(0, 2, 1),
                    ).astype(jnp.bfloat16))
    norm_arr = norm_flat.reshape(CORES, C_total, 128).transpose(0, 2, 1).copy()

    # per-(tile, blk): (global chunk list, group-local chunk list)
    tile_chunks = {}
    for g in groups:
        for tt in g["tiles"]:
            for bb in range(NBLK):
                lo0 = seg_chunk_start[tt, 0, bb]
                hi0 = seg_chunk_start[tt, 1, bb]
                cg = (list(range(lo0, lo0 + segchunks[tt, 0, bb]))
                      + list(range(hi0, hi0 + segchunks[tt, 1, bb])))
                tile_chunks[(tt, bb)] = cg

    return {
        "groups": groups,
        "tile_chunks": tile_chunks,
        "segchunks": segchunks,
        "C_total": C_total,
        "S": S,
        "idx_arr": idx_arr,
        "dcol_arr": dcol_arr,
        "norm_arr": norm_arr,
    }


# ablation switches for performance bisection (all True in production)
_FLAGS = {"gather": True, "mbuild": True, "mm": True, "epi": True, "cc": True,
          "mres": True, "fullmm": False, "compute": True}
# body replication count (timing only; >1 repeats the whole kernel in one NEFF)
_REPS = 1


def _build(meta):
    import concourse.bacc as bacc
    import concourse.mybir as mybir
    import concourse.tile as tile

    f32 = mybir.dt.float32
    bf16 = mybir.dt.bfloat16
    i16 = mybir.dt.int16

    C_total = meta["C_total"]
    S = meta["S"]
    groups = meta["groups"]
    tile_chunks = meta["tile_chunks"]

    nc = bacc.Bacc("TRN2", target_bir_lowering=False, debug=False,
                   enable_asserts=True, num_devices=CORES,
                   num_swdge_queues=NQ)

    xf = nc.dram_tensor("xf", [N, D], bf16, kind="ExternalInput")
    dlocb = nc.dram_tensor("dlocb", [128, C_total], bf16, kind="ExternalInput")
    normb = nc.dram_tensor("normb", [128, C_total], f32, kind="ExternalInput")
    idxb = nc.dram_tensor("idxb", [128, S // 16], i16, kind="ExternalInput")
    w1 = nc.dram_tensor("w1", [D, D], bf16, kind="ExternalInput")
    w2 = nc.dram_tensor("w2", [D, D], bf16, kind="ExternalInput")
    b1 = nc.dram_tensor("b1", [D, 1], f32, kind="ExternalInput")
    b2 = nc.dram_tensor("b2", [D, 1], f32, kind="ExternalInput")
    id32 = nc.dram_tensor("id32", [128, 128], f32, kind="ExternalInput")
    h1loc = nc.dram_tensor("h1loc", [NPC, D], bf16, kind="Internal")
    h1full = nc.dram_tensor("h1full", [N, D], bf16, kind="Internal",
                            addr_space="Shared")
    outp = nc.dram_tensor("outp", [NPC, D], f32, kind="ExternalOutput")

    gmax = max(g["glo"] + g["ghi"] for g in groups)
    qctr = [0]

    with tile.TileContext(nc) as tc:
        with (
            tc.tile_pool(name="const", bufs=1) as cpool,
            tc.tile_pool(name="gath", bufs=2) as gpool,
            tc.tile_pool(name="mmat", bufs=2) as mpool,
            tc.tile_pool(name="small", bufs=4) as spool,
            tc.tile_pool(name="rows", bufs=2) as rpool,
            tc.tile_pool(name="agg_ps", bufs=2, space="PSUM") as agg_ps,
            tc.tile_pool(name="out_ps", bufs=2, space="PSUM") as out_ps,
            tc.tile_pool(name="tr_ps", bufs=2, space="PSUM") as tr_ps,
        ):
            idx_t = cpool.tile([128, S // 16], i16, tag="idx")
            nc.sync.dma_start(idx_t[:], idxb.ap())
            dloc_t = cpool.tile([128, C_total], bf16, tag="dloc")
            nc.sync.dma_start(dloc_t[:], dlocb.ap())
            norm_t = cpool.tile([128, C_total], f32, tag="norm")
            nc.sync.dma_start(norm_t[:], normb.ap())
            w1_t = cpool.tile([D, D], bf16, tag="w1")
            nc.sync.dma_start(w1_t[:], w1.ap())
            w2_t = cpool.tile([D, D], bf16, tag="w2")
            nc.sync.dma_start(w2_t[:], w2.ap())
            b1_t = cpool.tile([D, 1], f32, tag="b1")
            nc.sync.dma_start(b1_t[:], b1.ap())
            b2_t = cpool.tile([D, 1], f32, tag="b2")
            nc.sync.dma_start(b2_t[:], b2.ap())
            id32_t = cpool.tile([128, 128], f32, tag="id32")
            nc.sync.dma_start(id32_t[:], id32.ap())
            # iota[p, 0, d] = d   (bf16; values < 32 are exact)
            iota_t = cpool.tile([128, 1, BLK], bf16, tag="iota")
            nc.gpsimd.iota(iota_t[:], pattern=[[0, 1], [1, BLK]],
                           channel_multiplier=0,
                           allow_small_or_imprecise_dtypes=True)
            # SBUF-resident scatter matrix, built once (shared by both layers):
            # m_t[p, c, d] = (d == dcol[p, c]) * norm[p, c]
            if _FLAGS["mres"]:
                m_t = cpool.tile([128, C_total, BLK], bf16, tag="mres")
                nc.vector.tensor_tensor(
                    m_t[:],
                    iota_t[:, 0:1, :].broadcast_to([128, C_total, BLK]),
                    dloc_t[:, :].unsqueeze(2).broadcast_to([128, C_total, BLK]),
                    op=mybir.AluOpType.is_equal)
                nc.vector.tensor_tensor(
                    m_t[:], m_t[:],
                    norm_t[:, :].unsqueeze(2).broadcast_to([128, C_total, BLK]),
                    op=mybir.AluOpType.mult)

            for _rep in range(_REPS):
              for layer in (1, 2):
                feat = xf if layer == 1 else h1full
                w_t = w1_t if layer == 1 else w2_t
                for g in groups:
                    glo, ghi = g["glo"], g["ghi"]
                    G = glo + ghi
                    c0 = g["c0"]
                    ntiles = len(g["tiles"])
                    gt = gpool.tile([128, gmax, D], bf16, tag="gt")
                    for lane, nch, base in ((0, glo, feat.ap()),
                                            (1, ghi, feat.ap()[SPLIT:N, :])):
                        if not _FLAGS["gather"]:
                            break
                        off = 0 if lane == 0 else glo
                        for cs in range(0, nch, MAXC):
                            cw = min(MAXC, nch - cs)
                            a = off + cs
                            nc.gpsimd.dma_gather(
                                gt[:, a:a + cw, :], base,
                                idx_t[:, (c0 + a) * 8:(c0 + a + cw) * 8],
                                num_idxs=cw * 128, num_idxs_reg=cw * 128,
                                elem_size=D,
                                queue_num=qctr[0] % NQ)
                            qctr[0] += 1
                    if not _FLAGS["mres"]:
                        m_t = mpool.tile([128, gmax, BLK], bf16, tag="mt")
                        nc.vector.tensor_tensor(
                            m_t[:, 0:G, :],
                            iota_t[:, 0:1, :].broadcast_to([128, G, BLK]),
                            dloc_t[:, c0:c0 + G].unsqueeze(2)
                                .broadcast_to([128, G, BLK]),
                            op=mybir.AluOpType.is_equal)
                        nc.vector.tensor_tensor(
                            m_t[:, 0:G, :], m_t[:, 0:G, :],
                            norm_t[:, c0:c0 + G].unsqueeze(2)
                                .broadcast_to([128, G, BLK]),
                            op=mybir.AluOpType.mult)
                        moff = c0
                    else:
                        moff = 0
                    if not _FLAGS["compute"]:
                        continue
                    rowt = rpool.tile([128, GROUP, D],
                                      bf16 if layer == 1 else f32,
                                      tag="hro" if layer == 1 else "oro")
                    for ti, tt in enumerate(g["tiles"]):
                        ps = agg_ps.tile([D, TILE], f32, tag="agg")
                        if _FLAGS["mm"]:
                            for bb in range(NBLK):
                                chunks = tile_chunks[(tt, bb)]
                                pcol = ps[:, bb * BLK:(bb + 1) * BLK]
                                for k, cg in enumerate(chunks):
                                    nc.tensor.matmul(
                                        pcol, gt[:, cg - c0, :],
                                        m_t[:, cg - moff, :],
                                        start=(k == 0),
                                        stop=(k == len(chunks) - 1))
                        elif _FLAGS["fullmm"]:
                            nc.tensor.matmul(ps[:], gt[:, 0, :], gt[:, 0, :],
                                             start=True, stop=True)
                        else:
                            for bb in range(NBLK):
                                nc.tensor.matmul(
                                    ps[:, bb * BLK:(bb + 1) * BLK],
                                    gt[:, 0, :], m_t[:, 0, :],
                                    start=True, stop=True)
                        aggT = spool.tile([D, TILE], bf16, tag="aggT")
                        nc.vector.tensor_copy(aggT[:], ps[:])
                        po = out_ps.tile([D, TILE], f32, tag="po")
                        nc.tensor.matmul(po[:], w_t[:], aggT[:],
                                         start=True, stop=True)
                        if layer == 1:
                            hT = spool.tile([D, TILE], f32, tag="hT")
                            nc.scalar.activation(
                                hT[:], po[:],
                                mybir.ActivationFunctionType.Relu,
                                bias=b1_t[:, 0:1], scale=1.0)
                            pt = tr_ps.tile([TILE, D], f32, tag="pt")
                            nc.tensor.transpose(pt[:], hT[:], id32_t[:])
                            nc.vector.tensor_copy(rowt[:, ti, :], pt[:])
                        else:
                            oT = spool.tile([D, TILE], f32, tag="oT")
                            nc.vector.tensor_scalar_add(oT[:], po[:],
                                                        b2_t[:, 0:1])
                            pt = tr_ps.tile([TILE, D], f32, tag="pt")
                            nc.tensor.transpose(pt[:], oT[:], id32_t[:])
                            nc.vector.tensor_copy(rowt[:, ti, :], pt[:])
                    # one batched store per group
                    r0 = g["tiles"][0] * TILE
                    rows = sum(TILE if tt < NT - 1 else LAST_ROWS
                               for tt in g["tiles"])
                    dstt = h1loc if layer == 1 else outp
                    srct = rowt
                    if rows == ntiles * TILE:
                        dap = dstt.ap()[r0:r0 + rows, :].rearrange(
                            "(t p) f -> p t f", p=TILE)
                        nc.sync.dma_start(dap, srct[:, 0:ntiles, :])
                    else:
                        # last group: full tiles batched, partial tile alone
                        nfull = ntiles - 1
                        if nfull:
                            dap = dstt.ap()[r0:r0 + nfull * TILE, :].rearrange(
                                "(t p) f -> p t f", p=TILE)
                            nc.sync.dma_start(dap, srct[:, 0:nfull, :])
                        pr0 = r0 + nfull * TILE
                        nc.sync.dma_start(
                            dstt.ap()[pr0:pr0 + LAST_ROWS, :],
                            srct[0:LAST_ROWS, nfull, :])
                if layer == 1 and _FLAGS["cc"]:
                    nc.gpsimd.collective_compute(
                        "AllGather", mybir.AluOpType.bypass,
                        replica_groups=[list(range(CORES))],
                        ins=[h1loc.ap()], outs=[h1full.ap()])
    nc.compile()
    return nc


class _Exec:
    """Device-resident SPMD executor mirroring bass2jax.run_bass_via_pjrt's
    multi-core branch, but caching the jitted callable and the device-resident
    input arrays so repeated runs skip re-trace and host->device transfer."""

    def __init__(self, nc):
        import jax
        import numpy as _np
        import concourse.mybir as mybir
        from concourse import bass2jax
        from jax.experimental.shard_map import shard_map
        from jax.sharding import Mesh, PartitionSpec

        bass2jax.install_neuronx_cc_hook()
        self.jax = jax
        self.nc = nc
        in_names, out_names, out_avals, zero_outs = [], [], [], []
        partition_name = (nc.partition_id_tensor.name
                          if nc.partition_id_tensor else None)
        for alloc in nc.m.functions[0].allocations:
            if not isinstance(alloc, mybir.MemoryLocationSet):
                continue
            name = alloc.memorylocations[0].name
            if alloc.kind == "ExternalInput":
                if name != partition_name:
                    in_names.append(name)
            elif alloc.kind == "ExternalOutput":
                out_names.append(name)
                shape = tuple(alloc.tensor_shape)
                dtype = mybir.dt.np(alloc.dtype)
                out_avals.append(jax.core.ShapedArray(shape, dtype))
                zero_outs.append(_np.zeros(shape, dtype))
        self.in_names, self.out_names = in_names, out_names
        self.out_avals, self.zero_outs = out_avals, zero_outs
        n_params, n_outs = len(in_names), len(out_names)
        all_names = list(in_names) + list(out_names)
        if partition_name is not None:
            all_names.append(partition_name)

        def _body(*args):
            ins = list(args[:n_params])
            outs = list(args[n_params:])
            operands = ins + outs
            if partition_name is not None:
                operands.append(bass2jax.partition_id_tensor())
            outs = list(bass2jax._bass_exec_p.bind(
                *operands,
                out_avals=tuple(out_avals),
                in_names=tuple(all_names),
                out_names=tuple(out_names),
                lowering_input_output_aliases=(),
                sim_require_finite=True,
                sim_require_nnan=True,
                nc=nc,
            ))
            return tuple(outs)

        devices = jax.devices()[:CORES]
        mesh = Mesh(_np.asarray(devices), ("core",))
        in_specs = (PartitionSpec("core"),) * (n_params + n_outs)
        out_specs = (PartitionSpec("core"),) * n_outs
        self.mesh = mesh
        self.sharded = jax.jit(
            shard_map(_body, mesh=mesh, in_specs=in_specs,
                      out_specs=out_specs, check_rep=False),
            donate_argnums=tuple(range(n_params, n_params + n_outs)),
            keep_unused=True,
        )
        self.dev_in = None

    def upload(self, in_maps):
        import jax
        import numpy as _np
        from jax.sharding import NamedSharding, PartitionSpec
        concat_in = [
            _np.concatenate([_np.asarray(in_maps[c][nm]) for c in range(CORES)],
                            axis=0)
            for nm in self.in_names
        ]
        sh = NamedSharding(self.mesh, PartitionSpec("core"))
        self.dev_in = [jax.device_put(a, sh) for a in concat_in]
        for a in self.dev_in:
            a.block_until_ready()

    def _zeros(self):
        import jax
        import numpy as _np
        from jax.sharding import NamedSharding, PartitionSpec
        sh = NamedSharding(self.mesh, PartitionSpec("core"))
        return [
            jax.device_put(
                _np.zeros((CORES * z.shape[0], *z.shape[1:]), z.dtype), sh)
            for z in self.zero_outs
        ]

    def run(self):
        import numpy as _np
        outs = self.sharded(*self.dev_in, *self._zeros())
        res = []
        for i, nm in enumerate(self.out_names):
            a = _np.asarray(outs[i]).reshape(CORES, *self.out_avals[i].shape)
            res.append(a)
        return dict(zip(self.out_names, res))

    def _time_burst(self, k, n):
        """Best wall over n trials of k back-to-back async executions with
        device-resident inputs and pre-uploaded donated output buffers."""
        import time as _t
        times = []
        for _ in range(n):
            zs_list = [self._zeros() for _ in range(k)]
            for zs in zs_list:
                for z in zs:
                    z.block_until_ready()
            t0 = _t.perf_counter()
            outs = [self.sharded(*self.dev_in, *zs) for zs in zs_list]
            for os_ in outs:
                for o in os_:
                    o.block_until_ready()
            times.append(_t.perf_counter() - t0)
        return min(times)


_CACHE = {}


def _in_maps(meta, x, W1, b1, W2, b2):
    import jax.numpy as jnp
    xf = np.asarray(jnp.asarray(np.asarray(x, dtype=np.float32)
                                ).astype(jnp.bfloat16))
    w1f = np.asarray(jnp.asarray(np.asarray(W1, dtype=np.float32)
                                 ).astype(jnp.bfloat16))
    w2f = np.asarray(jnp.asarray(np.asarray(W2, dtype=np.float32)
                                 ).astype(jnp.bfloat16))
    b1f = np.asarray(b1, dtype=np.float32).reshape(D, 1)
    b2f = np.asarray(b2, dtype=np.float32).reshape(D, 1)
    id32 = np.eye(128, dtype=np.float32)
    return [{
        "xf": xf,
        "dlocb": meta["dcol_arr"][c],
        "normb": meta["norm_arr"][c],
        "idxb": meta["idx_arr"][c],
        "w1": w1f, "w2": w2f, "b1": b1f, "b2": b2f,
        "id32": id32,
    } for c in range(CORES)]


def kernel(x, edge_index, W1, b1, W2, b2):
    meta = _prep(edge_index)
    nc = _build(meta)
    ex = _Exec(nc)
    ex.upload(_in_maps(meta, x, W1, b1, W2, b2))
    res = ex.run()
    _CACHE["exec"] = ex
    _CACHE["meta"] = meta
    out = res["outp"].reshape(N, D)
    return out.astype(np.float32)


def bench(n=4):
    """Differential per-exec time in ns: repeat the whole kernel body 5x
    inside a second NEFF and difference against the single-body NEFF, so
    dispatch/tunnel overhead cancels."""
    global _REPS
    meta = _CACHE["meta"]
    ex1 = _CACHE["exec"]
    w1 = ex1._time_burst(1, n + 2)
    old = _REPS
    try:
        _REPS = 5
        nc5 = _build(meta)
        ex5 = _Exec(nc5)
        ex5.dev_in = ex1.dev_in
        w5 = ex5._time_burst(1, n + 2)
    finally:
        _REPS = old
    return (w5 - w1) / 4 * 1e9


# revision 30
# speedup vs baseline: 29.1559x; 4.0590x over previous
"""Two-layer GCN encoder on 8 TRN2 NeuronCores.

Strategy (graph/data parallel, dst-sharded), v2:
  - Nodes partitioned contiguously across 8 cores (6250 dst rows each).
  - Features, weights, scatter matrices all bf16 (fp32 PSUM accumulate):
    halves gather/DMA traffic and quadruples PE throughput vs fp32.
  - Per layer:  agg.T[f, d] = sum_e gathered_feat[e, f] * M[e, d]  via PE
    matmuls over 128-edge chunks; M holds the GCN edge norm at the edge's
    local dst column (self-loops as explicit edges).  out.T = W.T @ agg.T,
    bias(/relu) epilogue, PE transpose back to row-major.
  - TILE=128 dst rows per PSUM tile (49 tiles/core), GROUP=8 tiles per
    gather batch.
  - M is built ON-CHIP (2 DVE ops per group): cmp = (iota == dloc_bcast),
    mt = cmp * norm_bcast, from tiny per-chunk dloc/norm blobs instead of
    DMAing the dense [128, C, TILE] scatter matrix from HBM.
  - Epilogue rows are batched per group: one store DMA per group instead of
    per tile (SP engine was 87% busy on per-tile stores).
  - Layer-1 activations are stored bf16 and AllGathered so every core holds
    the full feature table for layer 2's gathers.
  - dma_gather indices are int16, so each (tile, src-half) segment gathers
    from base row 0 or row 32768 of the feature table.
"""

import os
import sys
import numpy as np

for _p in ("/opt/trn_rl_repo", "/root/.axon_site/_ro/trn_rl_repo"):
    if os.path.isdir(_p) and _p not in sys.path:
        sys.path.insert(0, _p)

N = 50000
D = 128
CORES = 8
NPC = N // CORES            # 6250 dst rows per core
TILE = 128                  # dst rows per psum tile
NT = (NPC + TILE - 1) // TILE   # 49 tiles per core (last tile has 106 rows)
LAST_ROWS = NPC - (NT - 1) * TILE
SPLIT = 32768               # int16 gather-index base split
BLK = 32                    # dst columns per scatter-matmul block
NBLK = TILE // BLK          # 4 column blocks per psum tile
GROUP = 4                   # tiles per gather batch
MAXC = 8                    # chunks per dma_gather call (1024-descriptor ring)
NQ = 4                      # SWDGE queues used round-robin for gathers


def _prep(edge_index):
    """Sort/pad edges; build per-core gather-index and dcol/norm blobs.

    Segments are per (tile128, lane, 32-dst block): the chunk matmul
    accumulates into a 32-column slice of the [128,128] psum tile, so the
    SBUF-resident M matrix is only [128, C, 32]."""
    src = np.asarray(edge_index[0], dtype=np.int64)
    dst = np.asarray(edge_index[1], dtype=np.int64)
    deg = (np.bincount(dst, minlength=N) + 1).astype(np.float32)
    dinv = (1.0 / np.sqrt(deg)).astype(np.float32)

    loop = np.arange(N, dtype=np.int64)
    s_all = np.concatenate([src, loop])
    d_all = np.concatenate([dst, loop])
    norm = dinv[s_all] * dinv[d_all]

    core = d_all // NPC
    lcl = d_all - core * NPC
    t = lcl // TILE
    dloc = lcl - t * TILE
    blk = dloc // BLK
    lane = (s_all >= SPLIT).astype(np.int64)
    # fine segment key (core, tile, blk, lane)
    key = ((core * NT + t) * NBLK + blk) * 2 + lane

    order = np.argsort(key, kind="stable")
    key_s = key[order]
    s_s = s_all[order]
    norm_s = norm[order]
    dloc_s = dloc[order]
    lane_s = lane[order]
    core_s = core[order]

    counts = np.bincount(key, minlength=CORES * NT * NBLK * 2)
    counts = counts.reshape(CORES, NT, NBLK, 2)
    # two-level chunking: exact full chunks per (t, blk, lane); leftover edges
    # of the 4 blks share per-(t, lane) remainder chunks with 128-wide M
    fchunks = (counts // 128).max(axis=0)               # [NT, NBLK, 2]
    nfull_e = np.minimum(counts, fchunks[None] * 128)   # [CORES,NT,NBLK,2]
    rem_e = counts - nfull_e
    redges = rem_e.sum(axis=2)                          # [CORES, NT, 2]
    rchunks = ((redges + 127) // 128).max(axis=0)       # [NT, 2]
    rchunks[:, 0] = np.maximum(rchunks[:, 0], 1)  # >=1 rem chunk inits psum

    n_groups = (NT + GROUP - 1) // GROUP
    groups = []
    fstart = np.zeros((NT, NBLK, 2), dtype=np.int64)
    rstart = np.zeros((NT, 2), dtype=np.int64)
    c = 0
    for g in range(n_groups):
        ts = list(range(g * GROUP, min((g + 1) * GROUP, NT)))
        c0 = c
        for ll in (0, 1):
            for tt in ts:
                for bb in range(NBLK):
                    fstart[tt, bb, ll] = c
                    c += fchunks[tt, bb, ll]
            for tt in ts:
                rstart[tt, ll] = c
                c += rchunks[tt, ll]
            if ll == 0:
                glo = c - c0
        ghi = c - c0 - glo
        groups.append({"tiles": ts, "c0": c0, "glo": glo, "ghi": ghi})
    C_total = c
    S = C_total * 128

    # per-edge slot
    key_starts = np.zeros(CORES * NT * NBLK * 2 + 1, dtype=np.int64)
    np.cumsum(counts.reshape(-1), out=key_starts[1:])
    rank = np.arange(len(key_s)) - key_starts[key_s]
    k_core = key_s // (NT * NBLK * 2)
    k_rem = key_s - k_core * (NT * NBLK * 2)
    t_s = k_rem // (NBLK * 2)
    b_s = (k_rem // 2) % NBLK
    l_s = k_rem % 2
    nf = nfull_e[k_core, t_s, b_s, l_s]
    isfull = rank < nf
    # remainder rank: prefix of rem_e over blks within (core, t, lane)
    prevrem = np.cumsum(rem_e, axis=2) - rem_e          # exclusive prefix
    slot = np.where(
        isfull,
        fstart[t_s, b_s, l_s] * 128 + rank,
        rstart[t_s, l_s] * 128 + prevrem[k_core, t_s, b_s, l_s] + (rank - nf))

    idx_val = (s_s - lane_s * SPLIT).astype(np.int16)
    # pad slots: spread gather addresses over distinct (valid) rows instead of
    # hammering row 0 — their M entries are 0 so the data is discarded
    idx_flat = np.tile((np.arange(S, dtype=np.int64) % 16384).astype(np.int16),
                       (CORES, 1))
    idx_flat[core_s, slot] = idx_val
    # per-slot column value: dcol (0..31) for full chunks, dloc (0..127) for
    # remainder chunks; norm=0 on pad slots
    dv_flat = np.zeros((CORES, S), dtype=np.float32)
    dv_flat[core_s, slot] = np.where(isfull, dloc_s - b_s * BLK, dloc_s)
    norm_flat = np.zeros((CORES, S), dtype=np.float32)
    norm_flat[core_s, slot] = norm_s

    # chunk class masks + compact indices
    is_full_chunk = np.zeros(C_total, dtype=bool)
    for tt in range(NT):
        for bb in range(NBLK):
            for ll in (0, 1):
                is_full_chunk[fstart[tt, bb, ll]:
                              fstart[tt, bb, ll] + fchunks[tt, bb, ll]] = True
    cls_idx = np.zeros(C_total, dtype=np.int64)
    cls_idx[is_full_chunk] = np.arange(is_full_chunk.sum())
    cls_idx[~is_full_chunk] = np.arange((~is_full_chunk).sum())
    CF = int(is_full_chunk.sum())
    CR = C_total - CF

    # idx i lives at partition i%16 (replicated x8 across the 128 partitions)
    idx_arr = idx_flat.reshape(CORES, S // 16, 16).transpose(0, 2, 1)
    idx_arr = np.tile(idx_arr, (1, 8, 1)).copy()          # [CORES, 128, S//16]
    import jax.numpy as jnp
    dv3 = dv_flat.reshape(CORES, C_total, 128).transpose(0, 2, 1)
    nr3 = norm_flat.reshape(CORES, C_total, 128).transpose(0, 2, 1)
    dcolF = np.asarray(jnp.asarray(dv3[:, :, is_full_chunk]).astype(jnp.bfloat16))
    dlocR = np.asarray(jnp.asarray(dv3[:, :, ~is_full_chunk]).astype(jnp.bfloat16))
    normF = np.ascontiguousarray(nr3[:, :, is_full_chunk])
    normR = np.ascontiguousarray(nr3[:, :, ~is_full_chunk])

    # per-tile matmul op lists: [(global chunk, class idx)]
    tile_full = {}
    tile_rem = {}
    for tt in range(NT):
        for bb in range(NBLK):
            ops = []
            for ll in (0, 1):
                for cc in range(fstart[tt, bb, ll],
                                fstart[tt, bb, ll] + fchunks[tt, bb, ll]):
                    ops.append((cc, int(cls_idx[cc])))
            tile_full[(tt, bb)] = ops
        ops = []
        for ll in (0, 1):
            for cc in range(rstart[tt, ll], rstart[tt, ll] + rchunks[tt, ll]):
                ops.append((cc, int(cls_idx[cc])))
        tile_rem[tt] = ops

    return {
        "groups": groups,
        "tile_full": tile_full,
        "tile_rem": tile_rem,
        "C_total": C_total,
        "CF": CF,
        "CR": CR,
        "S": S,
        "idx_arr": idx_arr,
        "dcolF": dcolF,
        "dlocR": dlocR,
        "normF": normF,
        "normR": normR,
    }


# ablation switches for performance bisection (all True in production)
_FLAGS = {"gather": True, "mbuild": True, "mm": True, "epi": True, "cc": True,
          "mres": True, "fullmm": False, "compute": True}
# body replication count (timing only; >1 repeats the whole kernel in one NEFF)
_REPS = 1


def _build(meta):
    import concourse.bacc as bacc
    import concourse.mybir as mybir
    import concourse.tile as tile

    f32 = mybir.dt.float32
    bf16 = mybir.dt.bfloat16
    i16 = mybir.dt.int16

    C_total = meta["C_total"]
    CF, CR = meta["CF"], meta["CR"]
    S = meta["S"]
    groups = meta["groups"]
    tile_full = meta["tile_full"]
    tile_rem = meta["tile_rem"]

    nc = bacc.Bacc("TRN2", target_bir_lowering=False, debug=False,
                   enable_asserts=True, num_devices=CORES,
                   num_swdge_queues=NQ)

    xf = nc.dram_tensor("xf", [N, D], bf16, kind="ExternalInput")
    dcolFb = nc.dram_tensor("dcolFb", [128, CF], bf16, kind="ExternalInput")
    dlocRb = nc.dram_tensor("dlocRb", [128, CR], bf16, kind="ExternalInput")
    normFb = nc.dram_tensor("normFb", [128, CF], f32, kind="ExternalInput")
    normRb = nc.dram_tensor("normRb", [128, CR], f32, kind="ExternalInput")
    idxb = nc.dram_tensor("idxb", [128, S // 16], i16, kind="ExternalInput")
    w1 = nc.dram_tensor("w1", [D, D], bf16, kind="ExternalInput")
    w2 = nc.dram_tensor("w2", [D, D], bf16, kind="ExternalInput")
    b1 = nc.dram_tensor("b1", [D, 1], f32, kind="ExternalInput")
    b2 = nc.dram_tensor("b2", [D, 1], f32, kind="ExternalInput")
    id32 = nc.dram_tensor("id32", [128, 128], f32, kind="ExternalInput")
    h1loc = nc.dram_tensor("h1loc", [NPC, D], bf16, kind="Internal")
    h1full = nc.dram_tensor("h1full", [N, D], bf16, kind="Internal",
                            addr_space="Shared")
    outp = nc.dram_tensor("outp", [NPC, D], f32, kind="ExternalOutput")

    gmax = max(g["glo"] + g["ghi"] for g in groups)
    qctr = [0]

    with tile.TileContext(nc) as tc:
        with (
            tc.tile_pool(name="const", bufs=1) as cpool,
            tc.tile_pool(name="gath", bufs=2) as gpool,
            tc.tile_pool(name="small", bufs=4) as spool,
            tc.tile_pool(name="rows", bufs=2) as rpool,
            tc.tile_pool(name="agg_ps", bufs=2, space="PSUM") as agg_ps,
            tc.tile_pool(name="out_ps", bufs=2, space="PSUM") as out_ps,
            tc.tile_pool(name="tr_ps", bufs=2, space="PSUM") as tr_ps,
        ):
            idx_t = cpool.tile([128, S // 16], i16, tag="idx")
            nc.sync.dma_start(idx_t[:], idxb.ap())
            dcolF_t = cpool.tile([128, CF], bf16, tag="dcolF")
            nc.sync.dma_start(dcolF_t[:], dcolFb.ap())
            dlocR_t = cpool.tile([128, CR], bf16, tag="dlocR")
            nc.sync.dma_start(dlocR_t[:], dlocRb.ap())
            normF_t = cpool.tile([128, CF], f32, tag="normF")
            nc.sync.dma_start(normF_t[:], normFb.ap())
            normR_t = cpool.tile([128, CR], f32, tag="normR")
            nc.sync.dma_start(normR_t[:], normRb.ap())
            w1_t = cpool.tile([D, D], bf16, tag="w1")
            nc.sync.dma_start(w1_t[:], w1.ap())
            w2_t = cpool.tile([D, D], bf16, tag="w2")
            nc.sync.dma_start(w2_t[:], w2.ap())
            b1_t = cpool.tile([D, 1], f32, tag="b1")
            nc.sync.dma_start(b1_t[:], b1.ap())
            b2_t = cpool.tile([D, 1], f32, tag="b2")
            nc.sync.dma_start(b2_t[:], b2.ap())
            id32_t = cpool.tile([128, 128], f32, tag="id32")
            nc.sync.dma_start(id32_t[:], id32.ap())
            # iota[p, 0, d] = d   (bf16; values < 128 are exact)
            iota_t = cpool.tile([128, 1, BLK], bf16, tag="iota")
            nc.gpsimd.iota(iota_t[:], pattern=[[0, 1], [1, BLK]],
                           channel_multiplier=0,
                           allow_small_or_imprecise_dtypes=True)
            iotaR_t = cpool.tile([128, 1, TILE], bf16, tag="iotaR")
            nc.gpsimd.iota(iotaR_t[:], pattern=[[0, 1], [1, TILE]],
                           channel_multiplier=0,
                           allow_small_or_imprecise_dtypes=True)
            # SBUF-resident scatter matrices, built once (shared by both
            # layers): m[p, c, d] = (d == dcol[p, c]) * norm[p, c]
            m32 = cpool.tile([128, CF, BLK], bf16, tag="m32")
            nc.vector.tensor_tensor(
                m32[:],
                iota_t[:, 0:1, :].broadcast_to([128, CF, BLK]),
                dcolF_t[:, :].unsqueeze(2).broadcast_to([128, CF, BLK]),
                op=mybir.AluOpType.is_equal)
            nc.vector.tensor_tensor(
                m32[:], m32[:],
                normF_t[:, :].unsqueeze(2).broadcast_to([128, CF, BLK]),
                op=mybir.AluOpType.mult)
            m128 = cpool.tile([128, CR, TILE], bf16, tag="m128")
            nc.vector.tensor_tensor(
                m128[:],
                iotaR_t[:, 0:1, :].broadcast_to([128, CR, TILE]),
                dlocR_t[:, :].unsqueeze(2).broadcast_to([128, CR, TILE]),
                op=mybir.AluOpType.is_equal)
            nc.vector.tensor_tensor(
                m128[:], m128[:],
                normR_t[:, :].unsqueeze(2).broadcast_to([128, CR, TILE]),
                op=mybir.AluOpType.mult)

            for _rep in range(_REPS):
              for layer in (1, 2):
                feat = xf if layer == 1 else h1full
                w_t = w1_t if layer == 1 else w2_t
                for g in groups:
                    glo, ghi = g["glo"], g["ghi"]
                    G = glo + ghi
                    c0 = g["c0"]
                    ntiles = len(g["tiles"])
                    gt = gpool.tile([128, gmax, D], bf16, tag="gt")
                    for lane, nch, base in ((0, glo, feat.ap()),
                                            (1, ghi, feat.ap()[SPLIT:N, :])):
                        if not _FLAGS["gather"]:
                            break
                        off = 0 if lane == 0 else glo
                        for cs in range(0, nch, MAXC):
                            cw = min(MAXC, nch - cs)
                            a = off + cs
                            nc.gpsimd.dma_gather(
                                gt[:, a:a + cw, :], base,
                                idx_t[:, (c0 + a) * 8:(c0 + a + cw) * 8],
                                num_idxs=cw * 128, num_idxs_reg=cw * 128,
                                elem_size=D,
                                queue_num=qctr[0] % NQ)
                            qctr[0] += 1
                    if not _FLAGS["compute"]:
                        continue
                    rowt = rpool.tile([128, GROUP, D],
                                      bf16 if layer == 1 else f32,
                                      tag="hro" if layer == 1 else "oro")
                    for ti, tt in enumerate(g["tiles"]):
                        ps = agg_ps.tile([D, TILE], f32, tag="agg")
                        if _FLAGS["mm"]:
                            # remainder chunks first: 128-wide M initializes
                            # the whole psum tile (start=True on op 0)
                            rem = tile_rem[tt]
                            nfu = sum(len(tile_full[(tt, bb)])
                                      for bb in range(NBLK))
                            for k, (cg, ri) in enumerate(rem):
                                nc.tensor.matmul(
                                    ps[:], gt[:, cg - c0, :], m128[:, ri, :],
                                    start=(k == 0),
                                    stop=(k == len(rem) - 1 and nfu == 0),
                                    skip_group_check=True)
                            done = 0
                            for bb in range(NBLK):
                                pcol = ps[:, bb * BLK:(bb + 1) * BLK]
                                for cg, fi in tile_full[(tt, bb)]:
                                    done += 1
                                    nc.tensor.matmul(
                                        pcol, gt[:, cg - c0, :],
                                        m32[:, fi, :],
                                        start=False, stop=(done == nfu),
                                        skip_group_check=True)
                        else:
                            nc.tensor.matmul(ps[:], gt[:, 0, :],
                                             m128[:, 0, :],
                                             start=True, stop=True)
                        aggT = spool.tile([D, TILE], bf16, tag="aggT")
                        nc.vector.tensor_copy(aggT[:], ps[:])
                        po = out_ps.tile([D, TILE], f32, tag="po")
                        nc.tensor.matmul(po[:], w_t[:], aggT[:],
                                         start=True, stop=True)
                        if layer == 1:
                            hT = spool.tile([D, TILE], f32, tag="hT")
                            nc.scalar.activation(
                                hT[:], po[:],
                                mybir.ActivationFunctionType.Relu,
                                bias=b1_t[:, 0:1], scale=1.0)
                            pt = tr_ps.tile([TILE, D], f32, tag="pt")
                            nc.tensor.transpose(pt[:], hT[:], id32_t[:])
                            nc.vector.tensor_copy(rowt[:, ti, :], pt[:])
                        else:
                            oT = spool.tile([D, TILE], f32, tag="oT")
                            nc.vector.tensor_scalar_add(oT[:], po[:],
                                                        b2_t[:, 0:1])
                            pt = tr_ps.tile([TILE, D], f32, tag="pt")
                            nc.tensor.transpose(pt[:], oT[:], id32_t[:])
                            nc.vector.tensor_copy(rowt[:, ti, :], pt[:])
                    # one batched store per group
                    r0 = g["tiles"][0] * TILE
                    rows = sum(TILE if tt < NT - 1 else LAST_ROWS
                               for tt in g["tiles"])
                    dstt = h1loc if layer == 1 else outp
                    srct = rowt
                    if rows == ntiles * TILE:
                        dap = dstt.ap()[r0:r0 + rows, :].rearrange(
                            "(t p) f -> p t f", p=TILE)
                        nc.sync.dma_start(dap, srct[:, 0:ntiles, :])
                    else:
                        # last group: full tiles batched, partial tile alone
                        nfull = ntiles - 1
                        if nfull:
                            dap = dstt.ap()[r0:r0 + nfull * TILE, :].rearrange(
                                "(t p) f -> p t f", p=TILE)
                            nc.sync.dma_start(dap, srct[:, 0:nfull, :])
                        pr0 = r0 + nfull * TILE
                        nc.sync.dma_start(
                            dstt.ap()[pr0:pr0 + LAST_ROWS, :],
                            srct[0:LAST_ROWS, nfull, :])
                if layer == 1 and _FLAGS["cc"]:
                    nc.gpsimd.collective_compute(
                        "AllGather", mybir.AluOpType.bypass,
                        replica_groups=[list(range(CORES))],
                        ins=[h1loc.ap()], outs=[h1full.ap()])
    nc.compile()
    return nc


class _Exec:
    """Device-resident SPMD executor mirroring bass2jax.run_bass_via_pjrt's
    multi-core branch, but caching the jitted callable and the device-resident
    input arrays so repeated runs skip re-trace and host->device transfer."""

    def __init__(self, nc):
        import jax
        import numpy as _np
        import concourse.mybir as mybir
        from concourse import bass2jax
        from jax.experimental.shard_map import shard_map
        from jax.sharding import Mesh, PartitionSpec

        bass2jax.install_neuronx_cc_hook()
        self.jax = jax
        self.nc = nc
        in_names, out_names, out_avals, zero_outs = [], [], [], []
        partition_name = (nc.partition_id_tensor.name
                          if nc.partition_id_tensor else None)
        for alloc in nc.m.functions[0].allocations:
            if not isinstance(alloc, mybir.MemoryLocationSet):
                continue
            name = alloc.memorylocations[0].name
            if alloc.kind == "ExternalInput":
                if name != partition_name:
                    in_names.append(name)
            elif alloc.kind == "ExternalOutput":
                out_names.append(name)
                shape = tuple(alloc.tensor_shape)
                dtype = mybir.dt.np(alloc.dtype)
                out_avals.append(jax.core.ShapedArray(shape, dtype))
                zero_outs.append(_np.zeros(shape, dtype))
        self.in_names, self.out_names = in_names, out_names
        self.out_avals, self.zero_outs = out_avals, zero_outs
        n_params, n_outs = len(in_names), len(out_names)
        all_names = list(in_names) + list(out_names)
        if partition_name is not None:
            all_names.append(partition_name)

        def _body(*args):
            ins = list(args[:n_params])
            outs = list(args[n_params:])
            operands = ins + outs
            if partition_name is not None:
                operands.append(bass2jax.partition_id_tensor())
            outs = list(bass2jax._bass_exec_p.bind(
                *operands,
                out_avals=tuple(out_avals),
                in_names=tuple(all_names),
                out_names=tuple(out_names),
                lowering_input_output_aliases=(),
                sim_require_finite=True,
                sim_require_nnan=True,
                nc=nc,
            ))
            return tuple(outs)

        devices = jax.devices()[:CORES]
        mesh = Mesh(_np.asarray(devices), ("core",))
        in_specs = (PartitionSpec("core"),) * (n_params + n_outs)
        out_specs = (PartitionSpec("core"),) * n_outs
        self.mesh = mesh
        self.sharded = jax.jit(
            shard_map(_body, mesh=mesh, in_specs=in_specs,
                      out_specs=out_specs, check_rep=False),
            donate_argnums=tuple(range(n_params, n_params + n_outs)),
            keep_unused=True,
        )
        self.dev_in = None

    def upload(self, in_maps):
        import jax
        import numpy as _np
        from jax.sharding import NamedSharding, PartitionSpec
        concat_in = [
            _np.concatenate([_np.asarray(in_maps[c][nm]) for c in range(CORES)],
                            axis=0)
            for nm in self.in_names
        ]
        sh = NamedSharding(self.mesh, PartitionSpec("core"))
        self.dev_in = [jax.device_put(a, sh) for a in concat_in]
        for a in self.dev_in:
            a.block_until_ready()

    def _zeros(self):
        import jax
        import numpy as _np
        from jax.sharding import NamedSharding, PartitionSpec
        sh = NamedSharding(self.mesh, PartitionSpec("core"))
        return [
            jax.device_put(
                _np.zeros((CORES * z.shape[0], *z.shape[1:]), z.dtype), sh)
            for z in self.zero_outs
        ]

    def run(self):
        import numpy as _np
        outs = self.sharded(*self.dev_in, *self._zeros())
        res = []
        for i, nm in enumerate(self.out_names):
            a = _np.asarray(outs[i]).reshape(CORES, *self.out_avals[i].shape)
            res.append(a)
        return dict(zip(self.out_names, res))

    def _time_burst(self, k, n):
        """Best wall over n trials of k back-to-back async executions with
        device-resident inputs and pre-uploaded donated output buffers."""
        import time as _t
        times = []
        for _ in range(n):
            zs_list = [self._zeros() for _ in range(k)]
            for zs in zs_list:
                for z in zs:
                    z.block_until_ready()
            t0 = _t.perf_counter()
            outs = [self.sharded(*self.dev_in, *zs) for zs in zs_list]
            for os_ in outs:
                for o in os_:
                    o.block_until_ready()
            times.append(_t.perf_counter() - t0)
        return min(times)


_CACHE = {}


def _in_maps(meta, x, W1, b1, W2, b2):
    import jax.numpy as jnp
    xf = np.asarray(jnp.asarray(np.asarray(x, dtype=np.float32)
                                ).astype(jnp.bfloat16))
    w1f = np.asarray(jnp.asarray(np.asarray(W1, dtype=np.float32)
                                 ).astype(jnp.bfloat16))
    w2f = np.asarray(jnp.asarray(np.asarray(W2, dtype=np.float32)
                                 ).astype(jnp.bfloat16))
    b1f = np.asarray(b1, dtype=np.float32).reshape(D, 1)
    b2f = np.asarray(b2, dtype=np.float32).reshape(D, 1)
    id32 = np.eye(128, dtype=np.float32)
    return [{
        "xf": xf,
        "dcolFb": meta["dcolF"][c],
        "dlocRb": meta["dlocR"][c],
        "normFb": meta["normF"][c],
        "normRb": meta["normR"][c],
        "idxb": meta["idx_arr"][c],
        "w1": w1f, "w2": w2f, "b1": b1f, "b2": b2f,
        "id32": id32,
    } for c in range(CORES)]


def kernel(x, edge_index, W1, b1, W2, b2):
    meta = _prep(edge_index)
    nc = _build(meta)
    ex = _Exec(nc)
    ex.upload(_in_maps(meta, x, W1, b1, W2, b2))
    res = ex.run()
    _CACHE["exec"] = ex
    _CACHE["meta"] = meta
    out = res["outp"].reshape(N, D)
    return out.astype(np.float32)


def bench(n=4):
    """Differential per-exec time in ns: repeat the whole kernel body 5x
    inside a second NEFF and difference against the single-body NEFF, so
    dispatch/tunnel overhead cancels."""
    global _REPS
    meta = _CACHE["meta"]
    ex1 = _CACHE["exec"]
    w1 = ex1._time_burst(1, n + 2)
    old = _REPS
    try:
        _REPS = 5
        nc5 = _build(meta)
        ex5 = _Exec(nc5)
        ex5.dev_in = ex1.dev_in
        w5 = ex5._time_burst(1, n + 2)
    finally:
        _REPS = old
    return (w5 - w1) / 4 * 1e9
